# revision 15
# baseline (speedup 1.0000x reference)
"""Trainium2 Bass kernel for nn_AdjacencyMatrix (gnn_message_passing).

Reference computation:
    m = pad(x, [N, 1024]); repeat num_steps: m = 0.9 * (m @ W)
    y = m[:, -128:] * diag(W)[-128:]

Key algebraic collapse: only the first 256 columns of the padded state are
nonzero and only the last 128 output columns are read, so

    y = 0.9^k * x @ B,   B = (W^k)[0:256, -128:] * diag(W)[-128:]   (per col)

B is computed on-chip via the transposed chain T_i = ((W^i)[0:256, :]).T,
which uses W directly as the matmul stationary operand (no W transpose):

    T_1 = (W[0:256, :]).T            (16 PE tile transposes)
    T_{i+1} = W.T @ T_i              (f32r matmuls, 256-wide moving operand)
    T_k only needs row-tile 7 (cols 896:1024 of W^k)
    B = (diag-scaled T_k).T          (2 PE tile transposes)

Final: y = x @ B via PE with x transposed on-chip (f32r tile transposes).

Sharding: data-parallel over the batch dim N=16384 across 8 cores (2048 rows
per core); W replicated; no collectives.

Schedule notes:
  - W loads as 8 un-chained one-row-tile dma_starts alternating between the
    two HWDGE queues; per-queue FIFO keeps W fully ahead of x without
    completion-wait gaps between chunks (order pinned with sync=False deps).
  - x uses a batch-permuted SBUF layout [128p, 16t, 256d] (batch row =
    16p + t) so each DMA descriptor covers 16 KB contiguous HBM; the output
    store uses the matching [128p, 16t, 128c] layout (2 KB descriptors).
    The permutation is consistent end-to-end, so rows land at their
    original offsets.
  - Chain steps run a-major (outer loop over W row-tiles) so step 2 consumes
    W tiles as they arrive; two j-tiles share each PSUM bank via the
    documented has_written semantics (the pair's first MM start=True clears
    the bank; the sibling's first MM uses start=False and overwrites its
    cleared half).
  - A short burst of dummy PE ops at t=0 bridges the pre-W idle window.
  - The x transposes are emitted under tile_wait_until placement hints so
    the Tile scheduler cannot hoist them ahead of the chain, where their
    x-DMA wait would block the in-order PE queue (a global program-order
    dep chain over the PE stream was tried instead and cost ~20% steady
    matmul throughput by defeating LDWEIGHTS pull-ahead).
  - PSUM evictions alternate between DVE and ACT.
"""
import numpy as np

import concourse.bass as bass
import concourse.tile as tile
from concourse import bacc, mybir
from concourse.bass import _add_dep_helper
from concourse.bass_utils import run_bass_kernel_spmd
from concourse.masks import make_identity

F32 = mybir.dt.float32
F32R = mybir.dt.float32r

P = 128
N_ROWS = 16384
N_CORES = 8
ROWS_PER_CORE = N_ROWS // N_CORES  # 2048
D_IN = 256
N_NEURONS = 1024
N_OUT = 128
ENERGY_SCALAR = 0.9

NT = N_NEURONS // P  # 8 row/col tiles of W
DT = D_IN // P  # 2
TB = ROWS_PER_CORE // P  # 16 batch sub-rows per partition (t dim)


def build(num_steps: int) -> "bacc.Bacc":
    assert num_steps >= 1
    nc = bacc.Bacc("TRN2", target_bir_lowering=False, debug=False)

    x_d = nc.dram_tensor("x", [ROWS_PER_CORE, D_IN], F32R, kind="ExternalInput").ap()
    w_d = nc.dram_tensor(
        "weight", [N_NEURONS, N_NEURONS], F32R, kind="ExternalInput"
    ).ap()
    out_d = nc.dram_tensor(
        "out", [ROWS_PER_CORE, N_OUT], F32, kind="ExternalOutput"
    ).ap()

    # alternate PSUM evictions across the two elementwise engines
    _ev = [0]

    def evict(out_ap, in_ap):
        _ev[0] += 1
        if _ev[0] % 2:
            nc.vector.tensor_copy(out_ap, in_ap)
        else:
            nc.scalar.copy(out_ap, in_ap)

    with tile.TileContext(nc) as tc:
        with tc.tile_pool(name="persist", bufs=1) as pp, \
             tc.tile_pool(name="tp_ps", bufs=2, space="PSUM") as tp_ps, \
             tc.tile_pool(name="mm_ps", bufs=4, space="PSUM") as mm_ps, \
             tc.tile_pool(name="y_ps", bufs=2, space="PSUM") as y_ps:
            # identity gates every PE transpose
            ident_f = pp.tile([P, P], F32)
            make_identity(nc, ident_f)
            ident_r = pp.tile([P, P], F32R)
            nc.vector.tensor_copy(ident_r[:], ident_f[:])

            # ---- PE warm-up: dummy transposes, results never read; they
            # bridge the idle window before the first W tiles land ----
            for _ in range(7):
                ps_d = y_ps.tile([P, 4, N_OUT], F32R, tag="y", name="dummy")
                for i in range(4):
                    nc.tensor.transpose(ps_d[:, i, :], ident_r[:], ident_r[:])

            # ---- input DMAs (HWDGE, f32r end-to-end) ----
            w_sb = pp.tile([P, NT, N_NEURONS], F32R)
            x_sb = pp.tile([P, TB, D_IN], F32R)

            q_chain = {0: [], 1: []}  # queue -> [dma instrs in issue order]

            def q_push(qi, ins):
                chain = q_chain[qi]
                if chain:
                    _add_dep_helper(
                        ins.ins, chain[-1].ins, sync=False, reason="queue order"
                    )
                chain.append(ins)

            # the scalar/ACT queue starts ~1.5us after sync (its engine
            # preamble includes an activation-table load), so it carries
            # only 3 of the 8 W row-tiles; both queues then finish their W
            # share at about the same time, pulling the last-tile arrival
            # (which gates the chain) earlier
            for a in [0, 1, 2, 4, 6]:
                q_push(0, nc.sync.dma_start(
                    out=w_sb[:, a, :], in_=w_d[P * a : P * (a + 1), :]
                ))
            for a in [3, 5, 7]:
                q_push(1, nc.scalar.dma_start(
                    out=w_sb[:, a, :], in_=w_d[P * a : P * (a + 1), :]
                ))

            # x[16p + t, d] -> x_sb[p, t, d]: 16 KB per-partition descriptors
            x_r = bass.AP(
                tensor=x_d.tensor,
                offset=0,
                ap=[[TB * D_IN, P], [D_IN, TB], [1, D_IN]],
            )
            h = TB // 2
            for qi in range(2):
                eng = nc.sync if qi == 0 else nc.scalar
                q_push(qi, eng.dma_start(
                    out=x_sb[:, qi * h : (qi + 1) * h, :],
                    in_=x_r[:, qi * h : (qi + 1) * h, :],
                ))

            # diag(W)[-128:] -> [128, 1] via SWDGE (keeps the 4 B-element
            # descriptors off the HWDGE queues), scaled by 0.9^k
            diag_raw = pp.tile([P, 1], F32R)
            diag_ap = bass.AP(
                tensor=w_d.tensor,
                offset=(N_NEURONS - N_OUT) * N_NEURONS + (N_NEURONS - N_OUT),
                ap=[[N_NEURONS + 1, P], [1, 1]],
            )
            nc.gpsimd.dma_start(out=diag_raw[:], in_=diag_ap)
            diag_sc = pp.tile([P, 1], F32)
            nc.vector.tensor_scalar_mul(
                diag_sc[:], diag_raw[:], float(ENERGY_SCALAR**num_steps)
            )

            # ---- T_1 = (W[0:256, :]).T : [128, 8, 256] ----
            # T_1[:, a, 128t:128(t+1)] = (W_sb[:, t, 128a:128(a+1)]).T
            # needs only W row-tiles 0-1 (the first pair to arrive)
            def make_T1(a_tiles, dst):
                for gi in range(0, len(a_tiles), 2):  # 2 a-tiles per bank
                    aa = a_tiles[gi : gi + 2]
                    ps = tp_ps.tile([P, DT, DT, P], F32R, tag="tp")
                    for jj, a in enumerate(aa):
                        for t in range(DT):
                            nc.tensor.transpose(
                                ps[:, jj, t, :],
                                w_sb[:, t, P * a : P * (a + 1)],
                                ident_r[:],
                            )
                    evict(
                        dst[:, gi : gi + len(aa), :].rearrange(
                            "p j (t c) -> p j t c", t=DT
                        ),
                        ps[:, : len(aa), :, :],
                    )

            # ---- chain step, a-major so it rides the W DMA arrival ----
            # T_{i+1}[:, j, :] = sum_a (W_sb[:, a, 128j:]).T @ T_i[:, a, :]
            # accumulation order over a follows the expected W-tile
            # arrival order of the 5/3 queue split (sum over a commutes)
            A_ORDER = [0, 1, 3, 2, 5, 4, 7, 6]

            def chain_step(src, j_tiles, dst, scaled=False):
                banks = [
                    mm_ps.tile([P, 2, D_IN], F32, tag="mm", name=f"mm{bn}")
                    for bn in range((len(j_tiles) + 1) // 2)
                ]
                for pos, an in enumerate(A_ORDER):
                    for jn, j in enumerate(j_tiles):
                        nc.tensor.matmul(
                            banks[jn // 2][:, jn % 2, :],
                            lhsT=w_sb[:, an, P * j : P * (j + 1)],
                            rhs=src[:, an, :],
                            start=(pos == 0 and jn % 2 == 0),
                            stop=(pos == NT - 1),
                        )
                for bn, ps in enumerate(banks):
                    lo = 2 * bn
                    hi = min(lo + 2, len(j_tiles))
                    if scaled:
                        for jj in range(lo, hi):
                            nc.vector.tensor_scalar_mul(
                                dst[:, jj, :], ps[:, jj - lo, :], diag_sc[:]
                            )
                    else:
                        evict(dst[:, lo:hi, :], ps[:, : hi - lo, :])

            # ---- x transposes: xT[:, v, 128t + q] = x[16q + t, 128v + p] ----
            xT = pp.tile([P, DT, ROWS_PER_CORE], F32R)  # [128, 2, 2048]
            xt_emitted = set()

            def emit_xT_group(tq):  # tq indexes a pair of t values
                if tq in xt_emitted or tq >= TB // 2:
                    return
                xt_emitted.add(tq)
                ps = tp_ps.tile([P, DT, DT, P], F32R, tag="tp", name="xtp")
                for tt in range(2):
                    t = 2 * tq + tt
                    for v in range(DT):
                        nc.tensor.transpose(
                            ps[:, v, tt, :],
                            x_sb[:, t, P * v : P * (v + 1)],
                            ident_r[:],
                        )
                evict(
                    xT[:, :, 256 * tq : 256 * (tq + 1)].rearrange(
                        "p v (b c) -> p v b c", b=2
                    ),
                    ps[:, :, :, :],
                )

            T4 = pp.tile([P, 1, D_IN], F32R)  # scaled T_k row-tile 7
            if num_steps == 1:
                t1_last = pp.tile([P, 1, D_IN], F32R)
                make_T1([NT - 1], t1_last)
                nc.vector.tensor_scalar_mul(T4[:, 0, :], t1_last[:, 0, :], diag_sc[:])
            else:
                T_cur = pp.tile([P, NT, D_IN], F32R, name="T1")
                make_T1(list(range(NT)), T_cur)
                for step in range(2, num_steps):
                    T_nxt = pp.tile([P, NT, D_IN], F32R, name=f"T{step}")
                    chain_step(T_cur, list(range(NT)), T_nxt)
                    T_cur = T_nxt
                if num_steps >= 4:
                    # x has landed by the end of the second full chain step:
                    # weave the first x transposes into the T_3-eviction and
                    # T_4 bubbles so the PE stream stays dense.  The
                    # tile_wait_until placement hint stops the scheduler from
                    # hoisting them ahead of the chain, where their x-DMA
                    # wait would block the in-order PE queue.
                    with tc.tile_wait_until(0.027):
                        for tq in range(4):
                            emit_xT_group(tq)
                chain_step(T_cur, [NT - 1], T4, scaled=True)
                if num_steps >= 4:
                    with tc.tile_wait_until(0.0285):
                        for tq in range(4, 6):
                            emit_xT_group(tq)

            # ---- B = (T4).T : [128, 2, 128] f32r ----
            B_sb = pp.tile([P, DT, N_OUT], F32R)
            ps_b = tp_ps.tile([P, DT, DT, P], F32R, tag="tp")
            for u in range(DT):
                nc.tensor.transpose(
                    ps_b[:, 0, u, :], T4[:, 0, P * u : P * (u + 1)], ident_r[:]
                )
            nc.vector.tensor_copy(B_sb[:, 0, :], ps_b[:, 0, 0, :])
            nc.scalar.copy(B_sb[:, 1, :], ps_b[:, 0, 1, :])

            with tc.tile_wait_until(0.0295):
                for tq in range(TB // 2):
                    emit_xT_group(tq)

            # ---- y[16q + t, c] = sum_v xT[:, v, 128t + q].T @ B[:, v, :] ----
            y_sb = pp.tile([P, TB, N_OUT], F32)
            out_r = bass.AP(
                tensor=out_d.tensor,
                offset=0,
                ap=[[TB * N_OUT, P], [N_OUT, TB], [1, N_OUT]],
            )
            for g in range(4):
                ps = y_ps.tile([P, 4, N_OUT], F32, tag="y")
                for i in range(4):
                    t = 4 * g + i
                    for v in range(DT):
                        nc.tensor.matmul(
                            ps[:, i, :],
                            lhsT=xT[:, v, P * t : P * (t + 1)],
                            rhs=B_sb[:, v, :],
                            start=(v == 0),
                            stop=(v == DT - 1),
                        )
                if g == 3:
                    # last group: per-t evicts and stores across both
                    # engines/queues so the final store (and its HBM write
                    # receipt, which the exit barrier waits on) starts as
                    # early as possible
                    for i in range(4):
                        t = 4 * g + i
                        if i % 2 == 0:
                            nc.vector.tensor_copy(
                                y_sb[:, t : t + 1, :], ps[:, i : i + 1, :]
                            )
                        else:
                            nc.scalar.copy(
                                y_sb[:, t : t + 1, :], ps[:, i : i + 1, :]
                            )
                        eng = nc.sync if i % 2 == 0 else nc.scalar
                        q_push(i % 2, eng.dma_start(
                            out=out_r[:, t : t + 1, :],
                            in_=y_sb[:, t : t + 1, :],
                        ))
                else:
                    evict(y_sb[:, 4 * g : 4 * g + 4, :], ps[:, :, :])
                    qi = g % 2
                    eng = nc.sync if qi == 0 else nc.scalar
                    q_push(qi, eng.dma_start(
                        out=out_r[:, 4 * g : 4 * g + 4, :],
                        in_=y_sb[:, 4 * g : 4 * g + 4, :],
                    ))

    nc.compile()
    return nc


_NC_CACHE: dict = {}


def _get_nc(num_steps: int):
    if num_steps not in _NC_CACHE:
        _NC_CACHE[num_steps] = build(num_steps)
    return _NC_CACHE[num_steps]


def kernel(x: np.ndarray, weight: np.ndarray, num_steps) -> np.ndarray:
    k = int(num_steps)
    x = np.ascontiguousarray(x, dtype=np.float32)
    weight = np.ascontiguousarray(weight, dtype=np.float32)
    if k == 0:
        # pad(x)[:, -128:] is all zero (128 <= 1024 - 256)
        return np.zeros((x.shape[0], N_OUT), dtype=np.float32)

    nc = _get_nc(k)
    in_maps = [
        {
            "x": x[i * ROWS_PER_CORE : (i + 1) * ROWS_PER_CORE],
            "weight": weight,
        }
        for i in range(N_CORES)
    ]
    last_err = None
    for attempt in range(3):
        try:
            res = run_bass_kernel_spmd(nc, in_maps, core_ids=list(range(N_CORES)))
            return np.concatenate(
                [res.results[i]["out"] for i in range(N_CORES)], axis=0
            )
        except Exception as e:  # transient device wedges recover on retry
            last_err = e
            import time as _time

            _time.sleep(10)
    raise last_err


# revision 16
# speedup vs baseline: 1.0719x; 1.0719x over previous
"""Trainium2 Bass kernel for nn_AdjacencyMatrix (gnn_message_passing).

Reference computation:
    m = pad(x, [N, 1024]); repeat num_steps: m = 0.9 * (m @ W)
    y = m[:, -128:] * diag(W)[-128:]

Key algebraic collapse: only the first 256 columns of the padded state are
nonzero and only the last 128 output columns are read, so

    y = 0.9^k * x @ B,   B = (W^k)[0:256, -128:] * diag(W)[-128:]   (per col)

B is computed on-chip via the transposed chain T_i = ((W^i)[0:256, :]).T,
which uses W directly as the matmul stationary operand (no W transpose):

    T_1 = (W[0:256, :]).T            (16 PE tile transposes)
    T_{i+1} = W.T @ T_i              (f32r matmuls, 256-wide moving operand)
    T_k only needs row-tile 7 (cols 896:1024 of W^k)
    B = (diag-scaled T_k).T          (2 PE tile transposes)

Final: y = x @ B via PE with x transposed on-chip (f32r tile transposes).

Sharding: data-parallel over the batch dim N=16384 across 8 cores (2048 rows
per core); W replicated; no collectives.

Schedule notes:
  - W loads as 8 un-chained one-row-tile dma_starts alternating between the
    two HWDGE queues; per-queue FIFO keeps W fully ahead of x without
    completion-wait gaps between chunks (order pinned with sync=False deps).
  - x uses a batch-permuted SBUF layout [128p, 16t, 256d] (batch row =
    16p + t) so each DMA descriptor covers 16 KB contiguous HBM; the output
    store uses the matching [128p, 16t, 128c] layout (2 KB descriptors).
    The permutation is consistent end-to-end, so rows land at their
    original offsets.
  - Chain steps run a-major (outer loop over W row-tiles) so step 2 consumes
    W tiles as they arrive; two j-tiles share each PSUM bank via the
    documented has_written semantics (the pair's first MM start=True clears
    the bank; the sibling's first MM uses start=False and overwrites its
    cleared half).
  - A short burst of dummy PE ops at t=0 bridges the pre-W idle window.
  - The x transposes are emitted under tile_wait_until placement hints so
    the Tile scheduler cannot hoist them ahead of the chain, where their
    x-DMA wait would block the in-order PE queue (a global program-order
    dep chain over the PE stream was tried instead and cost ~20% steady
    matmul throughput by defeating LDWEIGHTS pull-ahead).
  - PSUM evictions alternate between DVE and ACT.
"""
import numpy as np

import concourse.bass as bass
import concourse.tile as tile
from concourse import bacc, mybir
from concourse.bass import _add_dep_helper
from concourse.bass_utils import run_bass_kernel_spmd
from concourse.masks import make_identity

F32 = mybir.dt.float32
F32R = mybir.dt.float32r

P = 128
N_ROWS = 16384
N_CORES = 8
ROWS_PER_CORE = N_ROWS // N_CORES  # 2048
D_IN = 256
N_NEURONS = 1024
N_OUT = 128
ENERGY_SCALAR = 0.9

NT = N_NEURONS // P  # 8 row/col tiles of W
DT = D_IN // P  # 2
TB = ROWS_PER_CORE // P  # 16 batch sub-rows per partition (t dim)


def build(num_steps: int) -> "bacc.Bacc":
    assert num_steps >= 1
    nc = bacc.Bacc("TRN2", target_bir_lowering=False, debug=False)

    x_d = nc.dram_tensor("x", [ROWS_PER_CORE, D_IN], F32R, kind="ExternalInput").ap()
    w_d = nc.dram_tensor(
        "weight", [N_NEURONS, N_NEURONS], F32R, kind="ExternalInput"
    ).ap()
    out_d = nc.dram_tensor(
        "out", [ROWS_PER_CORE, N_OUT], F32, kind="ExternalOutput"
    ).ap()

    # alternate PSUM evictions across the two elementwise engines
    _ev = [0]

    def evict(out_ap, in_ap):
        _ev[0] += 1
        if _ev[0] % 2:
            nc.vector.tensor_copy(out_ap, in_ap)
        else:
            nc.scalar.copy(out_ap, in_ap)

    with tile.TileContext(nc) as tc:
        with tc.tile_pool(name="persist", bufs=1) as pp, \
             tc.tile_pool(name="tp_ps", bufs=2, space="PSUM") as tp_ps, \
             tc.tile_pool(name="mm_ps", bufs=4, space="PSUM") as mm_ps, \
             tc.tile_pool(name="y_ps", bufs=2, space="PSUM") as y_ps:
            # identity gates every PE transpose
            ident_f = pp.tile([P, P], F32)
            make_identity(nc, ident_f)
            ident_r = pp.tile([P, P], F32R)
            nc.vector.tensor_copy(ident_r[:], ident_f[:])

            # ---- PE warm-up: dummy transposes, results never read; they
            # bridge the idle window before the first W tiles land ----
            for _ in range(7):
                ps_d = y_ps.tile([P, 4, N_OUT], F32R, tag="y", name="dummy")
                for i in range(4):
                    nc.tensor.transpose(ps_d[:, i, :], ident_r[:], ident_r[:])

            # ---- input DMAs (HWDGE, f32r end-to-end) ----
            w_sb = pp.tile([P, NT, N_NEURONS], F32R)
            x_sb = pp.tile([P, TB, D_IN], F32R)

            q_chain = {0: [], 1: []}  # queue -> [dma instrs in issue order]

            def q_push(qi, ins):
                chain = q_chain[qi]
                if chain:
                    _add_dep_helper(
                        ins.ins, chain[-1].ins, sync=False, reason="queue order"
                    )
                chain.append(ins)

            for a in range(NT):
                qi = a % 2
                eng = nc.sync if qi == 0 else nc.scalar
                q_push(qi, eng.dma_start(
                    out=w_sb[:, a, :], in_=w_d[P * a : P * (a + 1), :]
                ))

            # x[16p + t, d] -> x_sb[p, t, d]: 16 KB per-partition descriptors
            x_r = bass.AP(
                tensor=x_d.tensor,
                offset=0,
                ap=[[TB * D_IN, P], [D_IN, TB], [1, D_IN]],
            )
            h = TB // 2
            for qi in range(2):
                eng = nc.sync if qi == 0 else nc.scalar
                q_push(qi, eng.dma_start(
                    out=x_sb[:, qi * h : (qi + 1) * h, :],
                    in_=x_r[:, qi * h : (qi + 1) * h, :],
                ))

            # diag(W)[-128:] -> [128, 1] via SWDGE (keeps the 4 B-element
            # descriptors off the HWDGE queues), scaled by 0.9^k
            diag_raw = pp.tile([P, 1], F32R)
            diag_ap = bass.AP(
                tensor=w_d.tensor,
                offset=(N_NEURONS - N_OUT) * N_NEURONS + (N_NEURONS - N_OUT),
                ap=[[N_NEURONS + 1, P], [1, 1]],
            )
            nc.gpsimd.dma_start(out=diag_raw[:], in_=diag_ap)
            diag_sc = pp.tile([P, 1], F32)
            nc.vector.tensor_scalar_mul(
                diag_sc[:], diag_raw[:], float(ENERGY_SCALAR**num_steps)
            )

            # ---- T_1 = (W[0:256, :]).T : [128, 8, 256] ----
            # T_1[:, a, 128t:128(t+1)] = (W_sb[:, t, 128a:128(a+1)]).T
            # needs only W row-tiles 0-1 (the first pair to arrive)
            def make_T1(a_tiles, dst):
                for gi in range(0, len(a_tiles), 2):  # 2 a-tiles per bank
                    aa = a_tiles[gi : gi + 2]
                    ps = tp_ps.tile([P, DT, DT, P], F32R, tag="tp")
                    for jj, a in enumerate(aa):
                        for t in range(DT):
                            nc.tensor.transpose(
                                ps[:, jj, t, :],
                                w_sb[:, t, P * a : P * (a + 1)],
                                ident_r[:],
                            )
                    evict(
                        dst[:, gi : gi + len(aa), :].rearrange(
                            "p j (t c) -> p j t c", t=DT
                        ),
                        ps[:, : len(aa), :, :],
                    )

            # ---- chain step, a-major so it rides the W DMA arrival ----
            # T_{i+1}[:, j, :] = sum_a (W_sb[:, a, 128j:]).T @ T_i[:, a, :]
            def chain_step(src, j_tiles, dst, scaled=False):
                banks = [
                    mm_ps.tile([P, 2, D_IN], F32, tag="mm", name=f"mm{bn}")
                    for bn in range((len(j_tiles) + 1) // 2)
                ]
                for an in range(NT):
                    for jn, j in enumerate(j_tiles):
                        nc.tensor.matmul(
                            banks[jn // 2][:, jn % 2, :],
                            lhsT=w_sb[:, an, P * j : P * (j + 1)],
                            rhs=src[:, an, :],
                            start=(an == 0 and jn % 2 == 0),
                            stop=(an == NT - 1),
                        )
                for bn, ps in enumerate(banks):
                    lo = 2 * bn
                    hi = min(lo + 2, len(j_tiles))
                    if scaled:
                        for jj in range(lo, hi):
                            nc.vector.tensor_scalar_mul(
                                dst[:, jj, :], ps[:, jj - lo, :], diag_sc[:]
                            )
                    else:
                        evict(dst[:, lo:hi, :], ps[:, : hi - lo, :])

            # ---- x transposes: xT[:, v, 128t + q] = x[16q + t, 128v + p] ----
            xT = pp.tile([P, DT, ROWS_PER_CORE], F32R)  # [128, 2, 2048]
            xt_emitted = set()

            def emit_xT_group(tq):  # tq indexes a pair of t values
                if tq in xt_emitted or tq >= TB // 2:
                    return
                xt_emitted.add(tq)
                ps = tp_ps.tile([P, DT, DT, P], F32R, tag="tp", name="xtp")
                for tt in range(2):
                    t = 2 * tq + tt
                    for v in range(DT):
                        nc.tensor.transpose(
                            ps[:, v, tt, :],
                            x_sb[:, t, P * v : P * (v + 1)],
                            ident_r[:],
                        )
                evict(
                    xT[:, :, 256 * tq : 256 * (tq + 1)].rearrange(
                        "p v (b c) -> p v b c", b=2
                    ),
                    ps[:, :, :, :],
                )

            T4 = pp.tile([P, 1, D_IN], F32R)  # scaled T_k row-tile 7
            if num_steps == 1:
                t1_last = pp.tile([P, 1, D_IN], F32R)
                make_T1([NT - 1], t1_last)
                nc.vector.tensor_scalar_mul(T4[:, 0, :], t1_last[:, 0, :], diag_sc[:])
            else:
                T_cur = pp.tile([P, NT, D_IN], F32R, name="T1")
                make_T1(list(range(NT)), T_cur)
                for step in range(2, num_steps):
                    T_nxt = pp.tile([P, NT, D_IN], F32R, name=f"T{step}")
                    chain_step(T_cur, list(range(NT)), T_nxt)
                    T_cur = T_nxt
                if num_steps >= 4:
                    # x has landed by the end of the second full chain step:
                    # weave the first x transposes into the T_3-eviction and
                    # T_4 bubbles so the PE stream stays dense.  The
                    # tile_wait_until placement hint stops the scheduler from
                    # hoisting them ahead of the chain, where their x-DMA
                    # wait would block the in-order PE queue.
                    with tc.tile_wait_until(0.027):
                        for tq in range(4):
                            emit_xT_group(tq)
                chain_step(T_cur, [NT - 1], T4, scaled=True)
                if num_steps >= 4:
                    with tc.tile_wait_until(0.0285):
                        for tq in range(4, 6):
                            emit_xT_group(tq)

            # ---- B = (T4).T : [128, 2, 128] f32r ----
            B_sb = pp.tile([P, DT, N_OUT], F32R)
            ps_b = tp_ps.tile([P, DT, DT, P], F32R, tag="tp")
            for u in range(DT):
                nc.tensor.transpose(
                    ps_b[:, 0, u, :], T4[:, 0, P * u : P * (u + 1)], ident_r[:]
                )
            nc.vector.tensor_copy(B_sb[:, 0, :], ps_b[:, 0, 0, :])
            nc.scalar.copy(B_sb[:, 1, :], ps_b[:, 0, 1, :])

            with tc.tile_wait_until(0.0295):
                for tq in range(TB // 2):
                    emit_xT_group(tq)

            # ---- y[16q + t, c] = sum_v xT[:, v, 128t + q].T @ B[:, v, :] ----
            y_sb = pp.tile([P, TB, N_OUT], F32)
            out_r = bass.AP(
                tensor=out_d.tensor,
                offset=0,
                ap=[[TB * N_OUT, P], [N_OUT, TB], [1, N_OUT]],
            )
            for g in range(4):
                ps = y_ps.tile([P, 4, N_OUT], F32, tag="y")
                for i in range(4):
                    t = 4 * g + i
                    for v in range(DT):
                        nc.tensor.matmul(
                            ps[:, i, :],
                            lhsT=xT[:, v, P * t : P * (t + 1)],
                            rhs=B_sb[:, v, :],
                            start=(v == 0),
                            stop=(v == DT - 1),
                        )
                if g == 3:
                    # last group: split across both engines to shorten the
                    # kernel tail (eviction and store both halve)
                    nc.vector.tensor_copy(
                        y_sb[:, 4 * g : 4 * g + 2, :], ps[:, 0:2, :]
                    )
                    nc.scalar.copy(
                        y_sb[:, 4 * g + 2 : 4 * g + 4, :], ps[:, 2:4, :]
                    )
                    for hh in range(2):
                        eng = nc.sync if hh == 0 else nc.scalar
                        lo = 4 * g + 2 * hh
                        q_push(hh, eng.dma_start(
                            out=out_r[:, lo : lo + 2, :],
                            in_=y_sb[:, lo : lo + 2, :],
                        ))
                else:
                    evict(y_sb[:, 4 * g : 4 * g + 4, :], ps[:, :, :])
                    qi = g % 2
                    eng = nc.sync if qi == 0 else nc.scalar
                    q_push(qi, eng.dma_start(
                        out=out_r[:, 4 * g : 4 * g + 4, :],
                        in_=y_sb[:, 4 * g : 4 * g + 4, :],
                    ))

    nc.compile()
    return nc


_NC_CACHE: dict = {}


def _get_nc(num_steps: int):
    if num_steps not in _NC_CACHE:
        _NC_CACHE[num_steps] = build(num_steps)
    return _NC_CACHE[num_steps]


def kernel(x: np.ndarray, weight: np.ndarray, num_steps) -> np.ndarray:
    k = int(num_steps)
    x = np.ascontiguousarray(x, dtype=np.float32)
    weight = np.ascontiguousarray(weight, dtype=np.float32)
    if k == 0:
        # pad(x)[:, -128:] is all zero (128 <= 1024 - 256)
        return np.zeros((x.shape[0], N_OUT), dtype=np.float32)

    nc = _get_nc(k)
    in_maps = [
        {
            "x": x[i * ROWS_PER_CORE : (i + 1) * ROWS_PER_CORE],
            "weight": weight,
        }
        for i in range(N_CORES)
    ]
    last_err = None
    for attempt in range(3):
        try:
            res = run_bass_kernel_spmd(nc, in_maps, core_ids=list(range(N_CORES)))
            return np.concatenate(
                [res.results[i]["out"] for i in range(N_CORES)], axis=0
            )
        except Exception as e:  # transient device wedges recover on retry
            last_err = e
            import time as _time

            _time.sleep(10)
    raise last_err
